# revision 15
# baseline (speedup 1.0000x reference)
"""Trainium2 Bass kernel for a tiny per-pixel MLP (siren-style RGB net).

Network (from the reference):
    h = tanh(x @ W_in.T)            # [N, 8], x: [N, 2]
    h = tanh(h @ W_h.T)   (4x, shared weight)
    y = sigmoid(h @ W_out.T)        # [N, 3]

Strategy: pure data parallel over 8 NeuronCores (batch split). Per core the
pixel stream is processed in a "pack-16" layout: 16 pixels x 8 channels = 128
SBUF partitions, pixels streaming along the free dim. All matmuls run on the
tensor engine against host-built block-diagonal weight patterns; every
tanh/sigmoid is a single ACT instruction reading PSUM and writing SBUF (the
scalar engine is the roofline for this problem: 43 transcendental evals per
pixel at 1 elem/cycle/lane).

Layout details (per core, N=2097152 pixels):
  - x tile t = x_flat[65536*t : 65536*(t+1)] loaded as [128, 512] (partition u
    holds 512 consecutive floats = 256 consecutive pixels).
  - PE-transpose of each [128, 128] slice s gives xT[32q+2m+c, u] =
    x(pixel, c) with pixel(u,s,q,m) = 32768*t + 256*u + 64*s + 16*q + m.
  - Input layer: 4 row-strip matmuls (K=32, tile_position=(32q,0)) with
    lhsT[2m+c, 8m'+j] = delta(m,m') * W_in[j,c].
  - Hidden layers: K=128 matmuls with lhsT = block_diag_16(W_h.T).
  - Output layer: stationary operand = hidden-state chunk [128, 128], moving
    operand = block-diag W_out pattern [128, 48] -> output lands transposed:
    T[u, 3m+r], exactly the y memory order. Sigmoid writes a [128, 768]
    staging tile per x-tile that is one contiguous 384KB region of y.
"""

import numpy as np

import concourse.bass as bass
import concourse.mybir as mybir
import concourse.tile as tile
from concourse.bass_utils import run_bass_kernel_spmd

F32 = mybir.dt.float32
ACT = mybir.ActivationFunctionType

MAX_INST_WAITS = 1  # walrus CoreV3 setupSyncWait limit per instruction

N_PIXELS = 16777216
N_CORES = 8
N_CORE_PIX = N_PIXELS // N_CORES  # 2097152
TILE_PX = 32768                   # pixels per x-tile ([128, 512] floats)
GROUP = 4                         # x-tiles per input DMA (1 MiB)


def split_sem_waits(nc: bass.Bass, max_waits: int = MAX_INST_WAITS) -> int:
    """Split instructions carrying more than `max_waits` semaphore waits.

    The container's walrus rejects instructions with too many sync-wait
    commands ("Too many sync wait commands", CoreV3GenImpl setupSyncWait).
    Tile's end-of-kernel drain waits on the full global clock and can exceed
    the limit. Excess waits are moved onto NoOp instructions inserted just
    before the offender on the same engine — same-engine program order makes
    this semantically identical.
    """
    n_new = 0
    for f in nc.m.functions:
        for bb in f.blocks:
            insts = bb.instructions
            i = 0
            while i < len(insts):
                inst = insts[i]
                si = inst.sync_info
                if si is not None and si.on_wait and len(si.on_wait) > max_waits:
                    waits = list(si.on_wait)
                    keep = waits[-max_waits:]
                    extra = waits[:-max_waits]
                    for j in range(0, len(extra), max_waits):
                        chunk = extra[j : j + max_waits]
                        nop = mybir.InstNoOp(
                            name=f"I-waitsplit-{n_new}", ins=[], outs=[]
                        )
                        nop.engine = inst.engine
                        nop.sync_info = mybir.SyncInfo(on_wait=chunk, on_update=[])
                        nc.register_instruction(nop, overwrite=True)
                        insts.insert(i, nop)
                        i += 1
                        n_new += 1
                    si.on_wait = keep
                i += 1
    return n_new


def build_program(n_core_pix: int = N_CORE_PIX, repeats: int = 1) -> bass.Bass:
    """Build the per-core program. repeats>1 wraps the whole pixel sweep in a
    hardware For_i loop (for differential timing: the loop body re-reads the
    same inputs and rewrites the same outputs, so results are unchanged)."""
    n_tiles = n_core_pix // TILE_PX
    n_groups = (n_tiles + GROUP - 1) // GROUP
    assert n_tiles % GROUP == 0 and n_tiles * TILE_PX == n_core_pix

    nc = bass.Bass()

    x = nc.dram_tensor("x", [n_core_pix, 2], F32, kind="ExternalInput")
    w_in_pad_d = nc.dram_tensor("w_in_pad", [128, 512], F32, kind="ExternalInput")
    b_h_d = nc.dram_tensor("b_h", [128, 128], F32, kind="ExternalInput")
    w_out_d = nc.dram_tensor("w_out_blk", [128, 48], F32, kind="ExternalInput")
    ident_d = nc.dram_tensor("ident", [128, 128], F32, kind="ExternalInput")
    y = nc.dram_tensor("y", [n_core_pix, 3], F32, kind="ExternalOutput")

    # [g, u, t, (l c)]: partition u stride 512, tile t stride 65536, inner 512
    # contiguous floats.
    x_view = x.rearrange("(g t u l) c -> g u t (l c)", g=n_groups, t=GROUP, u=128, l=256)
    # [t, u, (w c)]: per tile one contiguous [128, 768] block of y.
    y_view = y.rearrange("(t u w) c -> t u (w c)", t=n_tiles, u=128, w=256)

    with tile.TileContext(nc) as tc:
        with (
            tc.tile_pool(name="consts", bufs=1) as cpool,
            tc.tile_pool(name="xin", bufs=3) as xpool,
            tc.tile_pool(name="xt", bufs=3) as xtpool,
            tc.tile_pool(name="z", bufs=6) as zpool,
            tc.tile_pool(name="stage", bufs=3) as stpool,
            tc.tile_pool(name="ps_layer", bufs=2, space="PSUM") as ps_layer,
            tc.tile_pool(name="ps_xt", bufs=2, space="PSUM") as ps_xt,
            tc.tile_pool(name="ps_t", bufs=2, space="PSUM") as ps_t,
        ):
            win_pad = cpool.tile([128, 512], F32)
            bh = cpool.tile([128, 128], F32)
            wout = cpool.tile([128, 48], F32)
            ident = cpool.tile([128, 128], F32)
            nc.sync.dma_start(out=win_pad[:], in_=w_in_pad_d[:])
            nc.sync.dma_start(out=bh[:], in_=b_h_d[:])
            nc.sync.dma_start(out=wout[:], in_=w_out_d[:])
            nc.sync.dma_start(out=ident[:], in_=ident_d[:])

            def sweep():
              for g in range(n_groups):
                xbuf = xpool.tile([128, GROUP, 512], F32)
                nc.sync.dma_start(out=xbuf[:], in_=x_view[g])

                for ti in range(GROUP):
                    t = g * GROUP + ti

                    # ---- transpose x into pack-16 layout ----
                    xt_ps = ps_xt.tile([128, 512], F32)
                    for s in range(4):
                        nc.tensor.transpose(
                            xt_ps[:, 128 * s : 128 * (s + 1)],
                            xbuf[:, ti, 128 * s : 128 * (s + 1)],
                            ident[:],
                        )
                    xt_sb = xtpool.tile([128, 512], F32)
                    nc.vector.tensor_copy(out=xt_sb[:], in_=xt_ps[:])

                    # ---- input layer: 4 row-strip matmuls per slice ----
                    z_half = []  # z tiles per half, one per layer
                    for h in range(2):
                        ps = ps_layer.tile([128, 1024], F32)
                        for s2 in range(2):
                            s = 2 * h + s2
                            for q in range(4):
                                nc.tensor.matmul(
                                    ps[:, 512 * s2 + 128 * q : 512 * s2 + 128 * (q + 1)],
                                    win_pad[:, 128 * q : 128 * (q + 1)],
                                    xt_sb[:, 128 * s : 128 * (s + 1)],
                                )
                        zh = zpool.tile([128, 1024], F32)
                        nc.scalar.activation(zh[:], ps[:], ACT.Tanh)
                        z_half.append(zh)

                    # ---- 4 hidden layers (shared weight) ----
                    for _layer in range(4):
                        new_half = []
                        for h in range(2):
                            ps = ps_layer.tile([128, 1024], F32)
                            for s2 in range(2):
                                nc.tensor.matmul(
                                    ps[:, 512 * s2 : 512 * (s2 + 1)],
                                    bh[:],
                                    z_half[h][:, 512 * s2 : 512 * (s2 + 1)],
                                )
                            zh = zpool.tile([128, 1024], F32)
                            nc.scalar.activation(zh[:], ps[:], ACT.Tanh)
                            new_half.append(zh)
                        z_half = new_half

                    # ---- output layer: hidden state stationary ----
                    st = stpool.tile([128, 768], F32)
                    for h in range(2):
                        tp = ps_t.tile([128, 384], F32)
                        for s2 in range(2):
                            for q in range(4):
                                c = 4 * s2 + q  # chunk within this half
                                nc.tensor.matmul(
                                    tp[:, 192 * s2 + 48 * q : 192 * s2 + 48 * (q + 1)],
                                    z_half[h][:, 128 * c : 128 * (c + 1)],
                                    wout[:],
                                )
                        # sigmoid(z) = 0.5*tanh(z/2) + 0.5 — keeps the kernel
                        # tanh-only (single ACT table set, no reload thrash);
                        # the affine runs on the otherwise-idle DVE.
                        nc.scalar.activation(
                            st[:, 384 * h : 384 * (h + 1)], tp[:], ACT.Tanh, scale=0.5
                        )
                        nc.vector.tensor_scalar(
                            st[:, 384 * h : 384 * (h + 1)],
                            st[:, 384 * h : 384 * (h + 1)],
                            0.5,
                            0.5,
                            mybir.AluOpType.mult,
                            mybir.AluOpType.add,
                        )
                    nc.sync.dma_start(out=y_view[t], in_=st[:])

            if repeats == 1:
                sweep()
            else:
                with tc.For_i(0, repeats, 1):
                    sweep()

    split_sem_waits(nc)
    return nc


def block_weights(W_in, W_h, W_out):
    """Host-side construction of the tiny block-diagonal weight patterns."""
    W_in = np.asarray(W_in, np.float32)
    W_h = np.asarray(W_h, np.float32)
    W_out = np.asarray(W_out, np.float32)

    w_in4 = np.zeros((128, 128), np.float32)
    for q in range(4):
        for m in range(16):
            # rows 32q+2m+c, cols 8m+j  ->  W_in[j, c]
            w_in4[32 * q + 2 * m : 32 * q + 2 * m + 2, 8 * m : 8 * m + 8] = W_in.T

    # Zero-padded per-strip input weights: full K=128 matmuls (zeros mask the
    # other strips) avoid tile_position row-strip matmuls entirely.
    w_in_pad = np.zeros((128, 512), np.float32)
    for q in range(4):
        w_in_pad[32 * q : 32 * (q + 1), 128 * q : 128 * q + 128] = w_in4[
            32 * q : 32 * (q + 1), :
        ]

    b_h = np.zeros((128, 128), np.float32)
    for m in range(16):
        # rows 8m+k, cols 8m+j  ->  W_h[j, k]
        b_h[8 * m : 8 * m + 8, 8 * m : 8 * m + 8] = W_h.T

    w_out_blk = np.zeros((128, 48), np.float32)
    for m in range(16):
        # rows 8m+j, cols 3m+r  ->  W_out[r, j]
        w_out_blk[8 * m : 8 * m + 8, 3 * m : 3 * m + 3] = W_out.T

    ident = np.eye(128, dtype=np.float32)
    return {
        "w_in_pad": w_in_pad,
        "b_h": b_h,
        "w_out_blk": w_out_blk,
        "ident": ident,
    }


def run(x, W_in, W_h, W_out, trace=False, n_cores=N_CORES):
    """Shard, execute on the 8 NeuronCores, gather. Returns (y, BassKernelResults)."""
    x = np.ascontiguousarray(x, np.float32)
    n = x.shape[0]
    per_core = n // n_cores
    nc = build_program(per_core)
    wmap = block_weights(W_in, W_h, W_out)
    in_maps = []
    for i in range(n_cores):
        m = dict(wmap)
        m["x"] = x[i * per_core : (i + 1) * per_core]
        in_maps.append(m)
    res = run_bass_kernel_spmd(nc, in_maps, list(range(n_cores)), trace=trace)
    y = np.concatenate([res.results[i]["y"] for i in range(n_cores)], axis=0)
    return y, res


def kernel(x, W_in, W_h, W_out):
    y, _ = run(x, W_in, W_h, W_out)
    return y


# revision 18
# speedup vs baseline: 1.3341x; 1.3341x over previous
"""Trainium2 Bass kernel for a tiny per-pixel MLP (siren-style RGB net).

Network (from the reference):
    h = tanh(x @ W_in.T)            # [N, 8], x: [N, 2]
    h = tanh(h @ W_h.T)   (4x, shared weight)
    y = sigmoid(h @ W_out.T)        # [N, 3]

Strategy: pure data parallel over 8 NeuronCores (batch split). Per core the
pixel stream is processed in a "pack-16" layout: 16 pixels x 8 channels = 128
SBUF partitions, pixels streaming along the free dim. All matmuls run on the
tensor engine against host-built block-diagonal weight patterns; every
tanh/sigmoid is a single ACT instruction reading PSUM and writing SBUF (the
scalar engine is the roofline for this problem: 43 transcendental evals per
pixel at 1 elem/cycle/lane).

Layout details (per core, N=2097152 pixels):
  - x tile t = x_flat[65536*t : 65536*(t+1)] loaded as [128, 512] (partition u
    holds 512 consecutive floats = 256 consecutive pixels).
  - PE-transpose of each [128, 128] slice s gives xT[32q+2m+c, u] =
    x(pixel, c) with pixel(u,s,q,m) = 32768*t + 256*u + 64*s + 16*q + m.
  - Input layer: 4 row-strip matmuls (K=32, tile_position=(32q,0)) with
    lhsT[2m+c, 8m'+j] = delta(m,m') * W_in[j,c].
  - Hidden layers: K=128 matmuls with lhsT = block_diag_16(W_h.T).
  - Output layer: stationary operand = hidden-state chunk [128, 128], moving
    operand = block-diag W_out pattern [128, 48] -> output lands transposed:
    T[u, 3m+r], exactly the y memory order. Sigmoid writes a [128, 768]
    staging tile per x-tile that is one contiguous 384KB region of y.
"""

import numpy as np

import concourse.bass as bass
import concourse.mybir as mybir
import concourse.tile as tile
from concourse.bass_utils import run_bass_kernel_spmd

F32 = mybir.dt.float32
F32R = mybir.dt.float32r  # single-pass reduced-precision fp32 matmul operand
ACT = mybir.ActivationFunctionType

MAX_INST_WAITS = 1  # walrus CoreV3 setupSyncWait limit per instruction

N_PIXELS = 16777216
N_CORES = 8
N_CORE_PIX = N_PIXELS // N_CORES  # 2097152
TILE_PX = 32768                   # pixels per x-tile ([128, 512] floats)
GROUP = 4                         # x-tiles per input DMA (1 MiB)


def split_sem_waits(nc: bass.Bass, max_waits: int = MAX_INST_WAITS) -> int:
    """Split instructions carrying more than `max_waits` semaphore waits.

    The container's walrus rejects instructions with too many sync-wait
    commands ("Too many sync wait commands", CoreV3GenImpl setupSyncWait).
    Tile's end-of-kernel drain waits on the full global clock and can exceed
    the limit. Excess waits are moved onto NoOp instructions inserted just
    before the offender on the same engine — same-engine program order makes
    this semantically identical.
    """
    n_new = 0
    for f in nc.m.functions:
        for bb in f.blocks:
            insts = bb.instructions
            i = 0
            while i < len(insts):
                inst = insts[i]
                si = inst.sync_info
                if si is not None and si.on_wait and len(si.on_wait) > max_waits:
                    waits = list(si.on_wait)
                    keep = waits[-max_waits:]
                    extra = waits[:-max_waits]
                    for j in range(0, len(extra), max_waits):
                        chunk = extra[j : j + max_waits]
                        nop = mybir.InstNoOp(
                            name=f"I-waitsplit-{n_new}", ins=[], outs=[]
                        )
                        nop.engine = inst.engine
                        nop.sync_info = mybir.SyncInfo(on_wait=chunk, on_update=[])
                        nc.register_instruction(nop, overwrite=True)
                        insts.insert(i, nop)
                        i += 1
                        n_new += 1
                    si.on_wait = keep
                i += 1
    return n_new


def build_program(n_core_pix: int = N_CORE_PIX, repeats: int = 1) -> bass.Bass:
    """Build the per-core program. repeats>1 wraps the whole pixel sweep in a
    hardware For_i loop (for differential timing: the loop body re-reads the
    same inputs and rewrites the same outputs, so results are unchanged)."""
    n_tiles = n_core_pix // TILE_PX
    n_groups = (n_tiles + GROUP - 1) // GROUP
    assert n_tiles % GROUP == 0 and n_tiles * TILE_PX == n_core_pix

    nc = bass.Bass()

    x = nc.dram_tensor("x", [n_core_pix, 2], F32, kind="ExternalInput")
    w_in_pad_d = nc.dram_tensor("w_in_pad", [128, 512], F32R, kind="ExternalInput")
    b_h_d = nc.dram_tensor("b_h", [128, 128], F32R, kind="ExternalInput")
    w_out_d = nc.dram_tensor("w_out_blk", [128, 48], F32R, kind="ExternalInput")
    ident_d = nc.dram_tensor("ident", [128, 128], F32, kind="ExternalInput")
    y = nc.dram_tensor("y", [n_core_pix, 3], F32, kind="ExternalOutput")

    # [g, u, t, (l c)]: partition u stride 512, tile t stride 65536, inner 512
    # contiguous floats.
    x_view = x.rearrange("(g t u l) c -> g u t (l c)", g=n_groups, t=GROUP, u=128, l=256)
    # [t, u, (w c)]: per tile one contiguous [128, 768] block of y.
    y_view = y.rearrange("(t u w) c -> t u (w c)", t=n_tiles, u=128, w=256)

    with tile.TileContext(nc) as tc:
        with (
            tc.tile_pool(name="consts", bufs=1) as cpool,
            tc.tile_pool(name="xin", bufs=3) as xpool,
            tc.tile_pool(name="xt", bufs=3) as xtpool,
            tc.tile_pool(name="z", bufs=6) as zpool,
            tc.tile_pool(name="stage", bufs=3) as stpool,
            tc.tile_pool(name="ps_layer", bufs=2, space="PSUM") as ps_layer,
            tc.tile_pool(name="ps_xt", bufs=2, space="PSUM") as ps_xt,
            tc.tile_pool(name="ps_t", bufs=2, space="PSUM") as ps_t,
        ):
            win_pad = cpool.tile([128, 512], F32R)
            bh = cpool.tile([128, 128], F32R)
            wout = cpool.tile([128, 48], F32R)
            ident = cpool.tile([128, 128], F32)
            nc.sync.dma_start(out=win_pad[:], in_=w_in_pad_d[:])
            nc.sync.dma_start(out=bh[:], in_=b_h_d[:])
            nc.sync.dma_start(out=wout[:], in_=w_out_d[:])
            nc.sync.dma_start(out=ident[:], in_=ident_d[:])

            def sweep():
              for g in range(n_groups):
                xbuf = xpool.tile([128, GROUP, 512], F32)
                nc.sync.dma_start(out=xbuf[:], in_=x_view[g])

                for ti in range(GROUP):
                    t = g * GROUP + ti

                    # ---- transpose x into pack-16 layout ----
                    xt_ps = ps_xt.tile([128, 512], F32)
                    for s in range(4):
                        nc.tensor.transpose(
                            xt_ps[:, 128 * s : 128 * (s + 1)],
                            xbuf[:, ti, 128 * s : 128 * (s + 1)],
                            ident[:],
                        )
                    xt_sb = xtpool.tile([128, 512], F32R)
                    nc.vector.tensor_copy(out=xt_sb[:], in_=xt_ps[:])

                    # ---- input layer: 4 row-strip matmuls per slice ----
                    z_half = []  # z tiles per half, one per layer
                    for h in range(2):
                        ps = ps_layer.tile([128, 1024], F32)
                        for s2 in range(2):
                            s = 2 * h + s2
                            for q in range(4):
                                nc.tensor.matmul(
                                    ps[:, 512 * s2 + 128 * q : 512 * s2 + 128 * (q + 1)],
                                    win_pad[:, 128 * q : 128 * (q + 1)],
                                    xt_sb[:, 128 * s : 128 * (s + 1)],
                                )
                        zh = zpool.tile([128, 1024], F32R)
                        nc.scalar.activation(zh[:], ps[:], ACT.Tanh)
                        z_half.append(zh)

                    # ---- 4 hidden layers (shared weight) ----
                    for _layer in range(4):
                        new_half = []
                        for h in range(2):
                            ps = ps_layer.tile([128, 1024], F32)
                            for s2 in range(2):
                                nc.tensor.matmul(
                                    ps[:, 512 * s2 : 512 * (s2 + 1)],
                                    bh[:],
                                    z_half[h][:, 512 * s2 : 512 * (s2 + 1)],
                                )
                            zh = zpool.tile([128, 1024], F32R)
                            nc.scalar.activation(zh[:], ps[:], ACT.Tanh)
                            new_half.append(zh)
                        z_half = new_half

                    # ---- output layer: hidden state stationary ----
                    st = stpool.tile([128, 768], F32)
                    for h in range(2):
                        tp = ps_t.tile([128, 384], F32)
                        for s2 in range(2):
                            for q in range(4):
                                c = 4 * s2 + q  # chunk within this half
                                nc.tensor.matmul(
                                    tp[:, 192 * s2 + 48 * q : 192 * s2 + 48 * (q + 1)],
                                    z_half[h][:, 128 * c : 128 * (c + 1)],
                                    wout[:],
                                )
                        # sigmoid(z) = 0.5*tanh(z/2) + 0.5 — keeps the kernel
                        # tanh-only (single ACT table set, no reload thrash);
                        # the affine runs on the otherwise-idle DVE.
                        nc.scalar.activation(
                            st[:, 384 * h : 384 * (h + 1)], tp[:], ACT.Tanh, scale=0.5
                        )
                        nc.vector.tensor_scalar(
                            st[:, 384 * h : 384 * (h + 1)],
                            st[:, 384 * h : 384 * (h + 1)],
                            0.5,
                            0.5,
                            mybir.AluOpType.mult,
                            mybir.AluOpType.add,
                        )
                    nc.sync.dma_start(out=y_view[t], in_=st[:])

            if repeats == 1:
                sweep()
            else:
                with tc.For_i(0, repeats, 1):
                    sweep()

    split_sem_waits(nc)
    return nc


def block_weights(W_in, W_h, W_out):
    """Host-side construction of the tiny block-diagonal weight patterns."""
    W_in = np.asarray(W_in, np.float32)
    W_h = np.asarray(W_h, np.float32)
    W_out = np.asarray(W_out, np.float32)

    w_in4 = np.zeros((128, 128), np.float32)
    for q in range(4):
        for m in range(16):
            # rows 32q+2m+c, cols 8m+j  ->  W_in[j, c]
            w_in4[32 * q + 2 * m : 32 * q + 2 * m + 2, 8 * m : 8 * m + 8] = W_in.T

    # Zero-padded per-strip input weights: full K=128 matmuls (zeros mask the
    # other strips) avoid tile_position row-strip matmuls entirely.
    w_in_pad = np.zeros((128, 512), np.float32)
    for q in range(4):
        w_in_pad[32 * q : 32 * (q + 1), 128 * q : 128 * q + 128] = w_in4[
            32 * q : 32 * (q + 1), :
        ]

    b_h = np.zeros((128, 128), np.float32)
    for m in range(16):
        # rows 8m+k, cols 8m+j  ->  W_h[j, k]
        b_h[8 * m : 8 * m + 8, 8 * m : 8 * m + 8] = W_h.T

    w_out_blk = np.zeros((128, 48), np.float32)
    for m in range(16):
        # rows 8m+j, cols 3m+r  ->  W_out[r, j]
        w_out_blk[8 * m : 8 * m + 8, 3 * m : 3 * m + 3] = W_out.T

    ident = np.eye(128, dtype=np.float32)
    return {
        "w_in_pad": w_in_pad,
        "b_h": b_h,
        "w_out_blk": w_out_blk,
        "ident": ident,
    }


def run(x, W_in, W_h, W_out, trace=False, n_cores=N_CORES):
    """Shard, execute on the 8 NeuronCores, gather. Returns (y, BassKernelResults)."""
    x = np.ascontiguousarray(x, np.float32)
    n = x.shape[0]
    per_core = n // n_cores
    nc = build_program(per_core)
    wmap = block_weights(W_in, W_h, W_out)
    in_maps = []
    for i in range(n_cores):
        m = dict(wmap)
        m["x"] = x[i * per_core : (i + 1) * per_core]
        in_maps.append(m)
    res = run_bass_kernel_spmd(nc, in_maps, list(range(n_cores)), trace=trace)
    y = np.concatenate([res.results[i]["y"] for i in range(n_cores)], axis=0)
    return y, res


def kernel(x, W_in, W_h, W_out):
    y, _ = run(x, W_in, W_h, W_out)
    return y
